# revision 7
# baseline (speedup 1.0000x reference)
"""Multi-head self-attention Trainium2 kernel (v2).

Problem: x[2, 2048, 768] -> MHSA (12 heads, head_dim 64) -> out[2, 2048, 768].

Sharding over 8 NeuronCores: core c handles batch c//4 and heads
[3*(c%4), 3*(c%4)+3). Each core computes its 3 heads' attention and a
row-split partial of the output projection over its 192 channels; the host
sums the 4 partials per batch and transposes.

v2 structure: queries are processed in two 1024-wide double-chunks so every
matmul stationary serves two 512-column moving passes; a pre-compile IR pass
then deletes the redundant second LDWEIGHTS of each pair (the Tile framework
emits one per matmul), removing ~100ns of serialized weight-load per matmul.
V stationaries are padded to 128 columns ([V|ones|0...]) to engage FWL.
The softmax exp is split between ScalarE (exact LUT exp) and VectorE
(Schraudolph fast-exp2: bits = round(s*C1 + C2) written as int16, bitcast to
bf16); the ~2% relative error on the DVE share is attenuated ~sqrt(neff)~27x
by softmax averaging. Row-sums ride the attn@V matmul as a ones column.
The output projection contracts h0/h1 as K=128 matmuls plus row-paired K=64
h2 matmuls (wp2 duplicated on partitions 64-127, m-even/odd pairs run
concurrently); it runs inside the second group loop of the last dchunk and
the epilogue. Matmul operands are bf16 (fp32 PSUM); normalization is fp32.
"""

import sys

sys.path.insert(0, "/opt/trn_rl_repo")

import numpy as np

EMBED = 768
N_SEQ = 2048
N_HEADS_CORE = 3
HD = 64
N_CORES = 8
KT = EMBED // 128  # 6 contraction tiles for the projections
MT = N_SEQ // 128  # 16 key-row tiles
QCH = 512  # PSUM-bank query chunk
DCH = 1024  # stationary-reuse double chunk
NDC = N_SEQ // DCH  # 2

# fast-exp2 constants: bf16 bits = trunc(s*0.125*log2e*128 + C2)
FEXP_C1 = 0.125 * 1.4426950408889634 * 128.0
FEXP_C2 = 127.0 * 128.0 + 0.5 - 5.5

# key-tile indices whose exp runs on VectorE (fast-exp2) instead of ScalarE
DVE_KT = {2, 5, 8, 11, 14}

_CACHED = {}


def _ldw_dedup(nc):
    """Delete InstLdweights whose weights are already resident.

    Tracks the last LDWEIGHTS per PE-array region (tile_position + extent);
    an LDWEIGHTS identical to the live one for a non-overwritten region is
    redundant. Dependency references to deleted instructions are remapped to
    the surviving load.
    """

    def ap_key(ins):
        ap = ins.ins[0]
        return (
            str(ap.memref),
            int(ap.offset),
            str(ap.ap),
            str(ap.dtype),
            str(ins.perf_mode),
            str(ins.is_transpose),
        )

    def extent(ins):
        pos = ins.tile_position
        if pos is None:
            return (0, 128, 0, 128)
        ap = ins.ins[0]
        try:
            pairs = list(ap.ap)
            rows = int(pairs[0][1])
            cols = int(pairs[-1][1]) if len(pairs) > 1 else 128
        except Exception:
            rows, cols = 128, 128
        r0, c0 = int(pos[0]), int(pos[1])
        return (r0, min(128, r0 + rows), c0, min(128, c0 + cols))

    removed = 0
    for f in nc.m.functions:
        for b in f.blocks:
            live = {}  # (r0,r1,c0,c1) -> (key, name)
            keep = []
            remap = {}
            for ins in b.instructions:
                if type(ins).__name__ == "InstLdweights":
                    k = ap_key(ins)
                    ext = extent(ins)
                    prev = live.get(ext)
                    if prev is not None and prev[0] == k:
                        remap[ins.name] = prev[1]
                        removed += 1
                        continue
                    # evict overlapping regions
                    r0, r1, c0, c1 = ext
                    for e in list(live):
                        if e != ext and not (
                            e[1] <= r0 or e[0] >= r1 or e[3] <= c0 or e[2] >= c1
                        ):
                            del live[e]
                    live[ext] = (k, ins.name)
                keep.append(ins)
            if remap:
                for ins in keep:
                    ins.remap_dependency_names(remap)
                b.instructions[:] = keep
    return removed


def _build():
    from concourse import bacc
    import concourse.tile as tile
    import concourse.mybir as mybir

    F32 = mybir.dt.float32
    BF16 = mybir.dt.bfloat16
    I16 = mybir.dt.int16
    EXP = mybir.ActivationFunctionType.Exp
    MULT = mybir.AluOpType.mult
    ADD = mybir.AluOpType.add

    nc = bacc.Bacc()
    xT = nc.declare_dram_parameter("xT", [EMBED, N_SEQ], BF16, isOutput=False)
    # Q/K weights in 3 m-tiles of 128 cols: [Qh0|Qh1], [Kh0|Kh1], [Qh2|Kh2]
    wqk = nc.declare_dram_parameter("wqk", [EMBED, 384], BF16, isOutput=False)
    wv = nc.declare_dram_parameter("wv", [EMBED, 192], BF16, isOutput=False)
    wp01 = nc.declare_dram_parameter("wp01", [128, EMBED], BF16, isOutput=False)
    # wp2 duplicated on both partition halves for row-paired h2 proj matmuls
    wp2b = nc.declare_dram_parameter("wp2b", [128, EMBED], BF16, isOutput=False)
    ones = nc.declare_dram_parameter("ones", [128, 3], BF16, isOutput=False)
    outT = nc.declare_dram_parameter("outT", [EMBED, N_SEQ], BF16, isOutput=True)

    with tile.TileContext(nc) as tc:
        with (
            tc.tile_pool(name="persist", bufs=1) as pp,
            tc.tile_pool(name="pt", bufs=36) as ptp,
            tc.tile_pool(name="work", bufs=5) as wk,
            tc.tile_pool(name="psS", bufs=2, space="PSUM") as psS,
            tc.tile_pool(name="psV", bufs=4, space="PSUM") as psV,
        ):
            qk = [
                pp.tile([128, N_SEQ], BF16, tag=f"qk{m}", name=f"qk{m}")
                for m in range(3)
            ]
            qk2d = pp.tile([128, N_SEQ], BF16, tag="qk2d")
            # vt[i]: per head h: cols [128h:128h+64]=V_h, col 128h+64=ones,
            # rest zero padding (128-col stationaries engage FWL)
            vt = [
                pp.tile([128, 384], BF16, tag=f"vt{m}", name=f"vt{m}")
                for m in range(MT)
            ]
            xt = [
                pp.tile([128, N_SEQ], BF16, tag=f"xt{k}", name=f"xt{k}")
                for k in range(KT)
            ]
            wqk_t = [
                pp.tile([128, 384], BF16, tag=f"wqk{k}", name=f"wqkt{k}")
                for k in range(KT)
            ]
            wv_t = [
                pp.tile([128, 192], BF16, tag=f"wv{k}", name=f"wvt{k}")
                for k in range(KT)
            ]
            wp01_t = pp.tile([128, EMBED], BF16, tag="wp01", name="wp01t")
            wp2b_t = pp.tile([128, EMBED], BF16, tag="wp2b", name="wp2bt")
            ao01 = pp.tile([128, N_SEQ], BF16, tag="ao01", name="ao01")
            # rows 0:64 = h2 attn out; rows 64:128 = duplicate (proj pairing)
            ao2b = pp.tile([128, N_SEQ], BF16, tag="ao2b", name="ao2b")

            xT_ap = xT[:, :].rearrange("(t p) n -> t p n", p=128)
            wqk_ap = wqk[:, :].rearrange("(t p) n -> t p n", p=128)
            wv_ap = wv[:, :].rearrange("(t p) n -> t p n", p=128)
            for k in range(KT):
                nc.gpsimd.dma_start(out=wqk_t[k], in_=wqk_ap[k])
            for c in range(NDC):
                cs = slice(c * DCH, (c + 1) * DCH)
                for k in range(KT):
                    nc.gpsimd.dma_start(out=xt[k][:, cs], in_=xT_ap[k][:, cs])
            for k in range(KT):
                nc.gpsimd.dma_start(out=wv_t[k], in_=wv_ap[k])
            nc.gpsimd.dma_start(out=wp01_t, in_=wp01[:, :])
            nc.gpsimd.dma_start(out=wp2b_t, in_=wp2b[:, :])
            for m in range(MT):
                nc.gpsimd.memset(vt[m], 0.0)

            # ---- HAM warm-up: PE busy from ~1us so the clock gate is at
            # K=8/8 when the real matmuls arrive (runs during input DMA).
            wsb = pp.tile([128, QCH], BF16, tag="wsb", name="wsb")
            nc.vector.memset(wsb, 0.0)
            warm = psV.tile([128, QCH], F32, tag="pv", name="warm")
            for _ in range(30):
                nc.tensor.matmul(
                    warm, wsb[:, 0:128], wsb, start=True, stop=True,
                )

            # ---- qk generation: per (m, dchunk): one stationary per k-tile
            # serves both 512-wide halves (second LDWEIGHTS deduped).
            def qk_mtile(m, d):
                ds = slice(d * DCH, (d + 1) * DCH)
                ps = psS.tile([128, DCH], F32, tag="sS", name="psqk")
                for k in range(KT):
                    for h in range(2):
                        hs = slice(d * DCH + h * QCH, d * DCH + (h + 1) * QCH)
                        nc.tensor.matmul(
                            ps[:, h * QCH : (h + 1) * QCH],
                            wqk_t[k][:, m * 128 : (m + 1) * 128],
                            xt[k][:, hs],
                            start=(k == 0),
                            stop=(k == KT - 1),
                        )
                nc.vector.tensor_copy(out=qk[m][:, ds], in_=ps)

            for m in (1, 0, 2):
                for d in range(NDC):
                    qk_mtile(m, d)
                if m == 2:
                    # [Qh2|Kh2] -> swapped copy [Kh2|Qh2]
                    nc.gpsimd.dma_start(out=qk2d[0:64, :], in_=qk[2][64:128, :])
                    nc.gpsimd.dma_start(out=qk2d[64:128, :], in_=qk[2][0:64, :])

            # ---- exp emission: ScalarE exact or VectorE fast-exp2 ----
            def exp_emit(pt, s, kt):
                if kt in DVE_KT:
                    nc.vector.tensor_scalar(
                        out=pt[:, :].bitcast(I16),
                        in0=s[:, :],
                        scalar1=FEXP_C1,
                        scalar2=FEXP_C2,
                        op0=MULT,
                        op1=ADD,
                    )
                else:
                    nc.scalar.activation(out=pt, in_=s, func=EXP, scale=0.125)

            # ---- scores for h0/h1: per key-tile, row-paired heads, both
            # q-halves on one stationary pair ----
            def scores01_kt(d, kt):
                ks = slice(kt * 128, (kt + 1) * 128)
                s0 = psS.tile([128, DCH], F32, tag="sS", name="s0")
                s1 = psS.tile([128, DCH], F32, tag="sS", name="s1")
                for h in range(2):
                    qs = slice(d * DCH + h * QCH, d * DCH + (h + 1) * QCH)
                    hs = slice(h * QCH, (h + 1) * QCH)
                    nc.tensor.matmul(
                        s0[:, hs], qk[1][0:64, ks], qk[0][0:64, qs],
                        start=True, stop=True, tile_position=(0, 0),
                    )
                    nc.tensor.matmul(
                        s1[:, hs], qk[1][64:128, ks], qk[0][64:128, qs],
                        start=True, stop=True, tile_position=(64, 0),
                    )
                pt0 = ptp.tile([128, DCH], BF16, tag="ptg", name="pt0")
                pt1 = ptp.tile([128, DCH], BF16, tag="ptg", name="pt1")
                exp_emit(pt0, s0, kt)
                exp_emit(pt1, s1, kt)
                return pt0, pt1

            # ---- scores for h2: two key-tiles row-paired via qk2d ----
            def scores2_2kt(d, g):
                i0, i1 = 2 * g, 2 * g + 1
                ksA = slice(i0 * 128, (i0 + 1) * 128)
                ksB = slice(i1 * 128, (i1 + 1) * 128)
                s2a = psS.tile([128, DCH], F32, tag="sS", name="s2a")
                s2b = psS.tile([128, DCH], F32, tag="sS", name="s2b")
                for h in range(2):
                    qs = slice(d * DCH + h * QCH, d * DCH + (h + 1) * QCH)
                    hs = slice(h * QCH, (h + 1) * QCH)
                    nc.tensor.matmul(
                        s2a[:, hs], qk2d[0:64, ksA], qk[2][0:64, qs],
                        start=True, stop=True, tile_position=(0, 0),
                    )
                    nc.tensor.matmul(
                        s2b[:, hs], qk[2][64:128, ksB], qk2d[64:128, qs],
                        start=True, stop=True, tile_position=(64, 0),
                    )
                pa = ptp.tile([128, DCH], BF16, tag="ptg", name="pt2a")
                pb = ptp.tile([128, DCH], BF16, tag="ptg", name="pt2b")
                exp_emit(pa, s2a, i0)
                exp_emit(pb, s2b, i1)
                return pa, pb

            # ---- attn@V for one head/key-tile: one stationary, both halves
            def attnv_kt(h, kt, pt, pvA, pvB):
                st = vt[kt][:, h * 128 : (h + 1) * 128]
                nc.tensor.matmul(
                    pvA, st, pt[:, 0:QCH],
                    start=(kt == 0), stop=(kt == MT - 1),
                )
                nc.tensor.matmul(
                    pvB, st, pt[:, QCH:],
                    start=(kt == 0), stop=(kt == MT - 1),
                )

            def softmax_divide(h, pv, qs):
                """Drain pv once (frees the PSUM bank), then normalize."""
                ov = wk.tile([65, QCH], F32, tag="ov", name="ov")
                nc.vector.tensor_copy(out=ov, in_=pv[0:65, :])
                # reciprocal with all 128 lanes: reshape [1,512] -> [128,4]
                rw = wk.tile([128, QCH // 128], F32, tag="rw", name="rw")
                nc.sync.dma_start(out=rw, in_=ov[64:65, :])
                nc.vector.reciprocal(out=rw, in_=rw)
                rs0 = wk.tile([1, QCH], F32, tag="rs0", name="rs0")
                nc.sync.dma_start(out=rs0, in_=rw)
                bc = wk.tile([64, QCH], F32, tag="bc", name="bc")
                nc.gpsimd.partition_broadcast(bc, rs0)
                if h == 0:
                    dst = ao01[0:64, qs]
                elif h == 1:
                    dst = ao01[64:128, qs]
                else:
                    dst = ao2b[0:64, qs]
                nc.vector.tensor_mul(out=dst, in0=ov[0:64, :], in1=bc)
                if h == 2:
                    # duplicate h2 rows onto partitions 64-127 for the
                    # row-paired proj matmuls (cross-partition -> DMA)
                    nc.gpsimd.dma_start(out=ao2b[64:128, qs], in_=ao2b[0:64, qs])

            # ---- V in natural layout [seq, ch] ----
            def vnat_mtile(m):
                ps = psV.tile([128, 192], F32, tag="pv", name="psv")
                for k in range(KT):
                    nc.tensor.matmul(
                        ps,
                        xt[k][:, m * 128 : (m + 1) * 128],
                        wv_t[k],
                        start=(k == 0),
                        stop=(k == KT - 1),
                    )
                for h in range(N_HEADS_CORE):
                    nc.vector.tensor_copy(
                        out=vt[m][:, 128 * h : 128 * h + 64],
                        in_=ps[:, 64 * h : 64 * h + 64],
                    )
                nc.gpsimd.dma_start(
                    out=vt[m].rearrange("p (h c) -> p h c", c=128)[:, :, 64],
                    in_=ones[:, :],
                )

            # ---- projection: per (m-pair, q-half): wp01 K=128 matmuls plus
            # row-paired wp2 K=64 matmuls for m-even/m-odd ----
            def proj_mpair(mp, qs):
                m0, m1 = 2 * mp, 2 * mp + 1
                out_ap = outT[:, :].rearrange("(t p) n -> t p n", p=128)
                po0 = psS.tile([128, QCH], F32, tag="sS", name="po0")
                po1 = psS.tile([128, QCH], F32, tag="sS", name="po1")
                nc.tensor.matmul(
                    po0, wp01_t[:, m0 * 128 : (m0 + 1) * 128], ao01[:, qs],
                    start=True, stop=False,
                )
                nc.tensor.matmul(
                    po1, wp01_t[:, m1 * 128 : (m1 + 1) * 128], ao01[:, qs],
                    start=True, stop=False,
                )
                nc.tensor.matmul(
                    po0, wp2b_t[0:64, m0 * 128 : (m0 + 1) * 128],
                    ao2b[0:64, qs],
                    start=False, stop=True, tile_position=(0, 0),
                )
                nc.tensor.matmul(
                    po1, wp2b_t[64:128, m1 * 128 : (m1 + 1) * 128],
                    ao2b[64:128, qs],
                    start=False, stop=True, tile_position=(64, 0),
                )
                for m, po in ((m0, po0), (m1, po1)):
                    ot = wk.tile([128, QCH], BF16, tag="ot", name="ot")
                    nc.vector.tensor_copy(out=ot, in_=po)
                    nc.sync.dma_start(out=out_ap[m][:, qs], in_=ot)

            # ---- initial phase: scores01(d0) interleaved with vnat ----
            pt01 = {}
            pt2 = {}
            for kt in range(MT):
                pt01[(0, kt)] = scores01_kt(0, kt)
                vnat_mtile(kt)

            # ---- main dchunk loop ----
            for d in range(NDC):
                qsA = slice(d * DCH, d * DCH + QCH)
                qsB = slice(d * DCH + QCH, (d + 1) * DCH)
                # loop 1: attnv for h0/h1 + scores2(d)
                pv0A = psV.tile([128, QCH], F32, tag="pv", name="pv0A")
                pv0B = psV.tile([128, QCH], F32, tag="pv", name="pv0B")
                pv1A = psV.tile([128, QCH], F32, tag="pv", name="pv1A")
                pv1B = psV.tile([128, QCH], F32, tag="pv", name="pv1B")
                for kt in range(MT):
                    p0, p1 = pt01.pop((d, kt))
                    attnv_kt(0, kt, p0, pv0A, pv0B)
                    attnv_kt(1, kt, p1, pv1A, pv1B)
                    if kt % 2 == 1:
                        g = kt // 2
                        pt2[(d, 2 * g)], pt2[(d, 2 * g + 1)] = scores2_2kt(d, g)
                softmax_divide(0, pv0A, qsA)
                softmax_divide(0, pv0B, qsB)
                softmax_divide(1, pv1A, qsA)
                softmax_divide(1, pv1B, qsB)

                # loop 2: attnv h2 + scores01(d+1) + proj(d-1)
                pv2A = psV.tile([128, QCH], F32, tag="pv", name="pv2A")
                pv2B = psV.tile([128, QCH], F32, tag="pv", name="pv2B")
                for kt in range(MT):
                    attnv_kt(2, kt, pt2.pop((d, kt)), pv2A, pv2B)
                    if d + 1 < NDC:
                        pt01[(d + 1, kt)] = scores01_kt(d + 1, kt)
                    if d == 1 and kt in (4, 9, 14):
                        mp = {4: 0, 9: 1, 14: 2}[kt]
                        proj_mpair(mp, slice(0 * DCH, 0 * DCH + QCH))
                        proj_mpair(mp, slice(0 * DCH + QCH, 1 * DCH))
                softmax_divide(2, pv2A, qsA)
                softmax_divide(2, pv2B, qsB)

            # keep the PE clock gate open across the divide chain that
            # precedes the epilogue projection
            tailw = psS.tile([128, QCH], F32, tag="sS", name="tailw")
            for _ in range(8):
                nc.tensor.matmul(
                    tailw, wsb[:, 0:128], wsb, start=True, stop=True,
                )

            # epilogue: last dchunk's projection
            for mp in range(3):
                proj_mpair(mp, slice(1 * DCH, 1 * DCH + QCH))
                proj_mpair(mp, slice(1 * DCH + QCH, 2 * DCH))

    ndel = _ldw_dedup(nc)
    nc.compile()
    nc._ldw_deduped = ndel
    return nc


def _get_nc():
    if "nc" not in _CACHED:
        _CACHED["nc"] = _build()
    return _CACHED["nc"]


def _shard_inputs(x, w_qkv, w_proj):
    """Build the 8 per-core input maps (bf16 operands)."""
    import ml_dtypes

    bf = ml_dtypes.bfloat16
    in_maps = []
    for core in range(N_CORES):
        b = core // 4
        h0 = 3 * (core % 4)
        heads = [h0, h0 + 1, h0 + 2]
        xTc = np.ascontiguousarray(x[b].T).astype(bf)
        wq = [w_qkv[:, h * HD : (h + 1) * HD] for h in heads]
        wk_ = [w_qkv[:, EMBED + h * HD : EMBED + (h + 1) * HD] for h in heads]
        wv_ = [
            w_qkv[:, 2 * EMBED + h * HD : 2 * EMBED + (h + 1) * HD] for h in heads
        ]
        wqk = np.concatenate(
            [wq[0], wq[1], wk_[0], wk_[1], wq[2], wk_[2]], axis=1
        ).astype(bf)
        wvp = np.concatenate([wv_[0], wv_[1], wv_[2]], axis=1).astype(bf)
        wps = [
            np.ascontiguousarray(w_proj[h * HD : (h + 1) * HD, :]).astype(bf)
            for h in heads
        ]
        wp2b = np.concatenate([wps[2], wps[2]], axis=0)
        in_maps.append(
            {
                "ones": np.ones((128, 3), bf),
                "xT": xTc,
                "wqk": np.ascontiguousarray(wqk),
                "wv": np.ascontiguousarray(wvp),
                "wp01": np.ascontiguousarray(np.concatenate([wps[0], wps[1]], axis=0)),
                "wp2b": np.ascontiguousarray(wp2b),
            }
        )
    return in_maps


def kernel(x, w_qkv, w_proj, _trace=False):
    from concourse.bass_utils import run_bass_kernel_spmd

    x = np.asarray(x, dtype=np.float32)
    w_qkv = np.asarray(w_qkv, dtype=np.float32)
    w_proj = np.asarray(w_proj, dtype=np.float32)

    nc = _get_nc()
    in_maps = _shard_inputs(x, w_qkv, w_proj)
    res = run_bass_kernel_spmd(
        nc, in_maps, core_ids=list(range(N_CORES)), trace=_trace
    )
    _CACHED["last_results"] = res

    out = np.empty((2, N_SEQ, EMBED), dtype=np.float32)
    for b in range(2):
        acc = res.results[4 * b]["outT"].astype(np.float32).copy()
        for g in range(1, 4):
            acc += res.results[4 * b + g]["outT"].astype(np.float32)
        out[b] = acc.T
    return out


# revision 8
# speedup vs baseline: 1.0071x; 1.0071x over previous
"""Multi-head self-attention Trainium2 kernel (v2).

Problem: x[2, 2048, 768] -> MHSA (12 heads, head_dim 64) -> out[2, 2048, 768].

Sharding over 8 NeuronCores: core c handles batch c//4 and heads
[3*(c%4), 3*(c%4)+3). Each core computes its 3 heads' attention and a
row-split partial of the output projection over its 192 channels; the host
sums the 4 partials per batch and transposes.

v2 structure: queries are processed in two 1024-wide double-chunks so every
matmul stationary serves two 512-column moving passes; a pre-compile IR pass
then deletes the redundant second LDWEIGHTS of each pair (the Tile framework
emits one per matmul), removing ~100ns of serialized weight-load per matmul.
V stationaries are padded to 128 columns ([V|ones|0...]) to engage FWL.
The softmax exp is split between ScalarE (exact LUT exp) and VectorE
(Schraudolph fast-exp2: bits = round(s*C1 + C2) written as int16, bitcast to
bf16); the ~2% relative error on the DVE share is attenuated ~sqrt(neff)~27x
by softmax averaging. Row-sums ride the attn@V matmul as a ones column.
The output projection contracts h0/h1 as K=128 matmuls plus row-paired K=64
h2 matmuls (wp2 duplicated on partitions 64-127, m-even/odd pairs run
concurrently); it runs inside the second group loop of the last dchunk and
the epilogue. Matmul operands are bf16 (fp32 PSUM); normalization is fp32.
"""

import sys

sys.path.insert(0, "/opt/trn_rl_repo")

import numpy as np

EMBED = 768
N_SEQ = 2048
N_HEADS_CORE = 3
HD = 64
N_CORES = 8
KT = EMBED // 128  # 6 contraction tiles for the projections
MT = N_SEQ // 128  # 16 key-row tiles
QCH = 512  # PSUM-bank query chunk
DCH = 1024  # stationary-reuse double chunk
NDC = N_SEQ // DCH  # 2

# fast-exp2 constants: bf16 bits = trunc(s*0.125*log2e*128 + C2)
FEXP_C1 = 0.125 * 1.4426950408889634 * 128.0
FEXP_C2 = 127.0 * 128.0 + 0.5 - 5.5

_CACHED = {}


def _ldw_dedup(nc):
    """Delete InstLdweights whose weights are already resident.

    Tracks the last LDWEIGHTS per PE-array region (tile_position + extent);
    an LDWEIGHTS identical to the live one for a non-overwritten region is
    redundant. Dependency references to deleted instructions are remapped to
    the surviving load.
    """

    def ap_key(ins):
        ap = ins.ins[0]
        return (
            str(ap.memref),
            int(ap.offset),
            str(ap.ap),
            str(ap.dtype),
            str(ins.perf_mode),
            str(ins.is_transpose),
        )

    def extent(ins):
        pos = ins.tile_position
        if pos is None:
            return (0, 128, 0, 128)
        ap = ins.ins[0]
        try:
            pairs = list(ap.ap)
            rows = int(pairs[0][1])
            cols = int(pairs[-1][1]) if len(pairs) > 1 else 128
        except Exception:
            rows, cols = 128, 128
        r0, c0 = int(pos[0]), int(pos[1])
        return (r0, min(128, r0 + rows), c0, min(128, c0 + cols))

    removed = 0
    for f in nc.m.functions:
        for b in f.blocks:
            live = {}  # (r0,r1,c0,c1) -> (key, name)
            keep = []
            remap = {}
            for ins in b.instructions:
                if type(ins).__name__ == "InstLdweights":
                    k = ap_key(ins)
                    ext = extent(ins)
                    prev = live.get(ext)
                    if prev is not None and prev[0] == k:
                        remap[ins.name] = prev[1]
                        removed += 1
                        continue
                    # evict overlapping regions
                    r0, r1, c0, c1 = ext
                    for e in list(live):
                        if e != ext and not (
                            e[1] <= r0 or e[0] >= r1 or e[3] <= c0 or e[2] >= c1
                        ):
                            del live[e]
                    live[ext] = (k, ins.name)
                keep.append(ins)
            if remap:
                for ins in keep:
                    ins.remap_dependency_names(remap)
                b.instructions[:] = keep
    return removed


def _build():
    from concourse import bacc
    import concourse.tile as tile
    import concourse.mybir as mybir

    F32 = mybir.dt.float32
    BF16 = mybir.dt.bfloat16
    I16 = mybir.dt.int16
    EXP = mybir.ActivationFunctionType.Exp
    MULT = mybir.AluOpType.mult
    ADD = mybir.AluOpType.add

    nc = bacc.Bacc()
    xT = nc.declare_dram_parameter("xT", [EMBED, N_SEQ], BF16, isOutput=False)
    # Q/K weights in 3 m-tiles of 128 cols: [Qh0|Qh1], [Kh0|Kh1], [Qh2|Kh2]
    wqk = nc.declare_dram_parameter("wqk", [EMBED, 384], BF16, isOutput=False)
    wv = nc.declare_dram_parameter("wv", [EMBED, 192], BF16, isOutput=False)
    wp01 = nc.declare_dram_parameter("wp01", [128, EMBED], BF16, isOutput=False)
    # wp2 duplicated on both partition halves for row-paired h2 proj matmuls
    wp2b = nc.declare_dram_parameter("wp2b", [128, EMBED], BF16, isOutput=False)
    ones = nc.declare_dram_parameter("ones", [128, 3], BF16, isOutput=False)
    outT = nc.declare_dram_parameter("outT", [EMBED, N_SEQ], BF16, isOutput=True)

    with tile.TileContext(nc) as tc:
        with (
            tc.tile_pool(name="persist", bufs=1) as pp,
            tc.tile_pool(name="pt", bufs=36) as ptp,
            tc.tile_pool(name="work", bufs=5) as wk,
            tc.tile_pool(name="psS", bufs=2, space="PSUM") as psS,
            tc.tile_pool(name="psV", bufs=4, space="PSUM") as psV,
        ):
            qk = [
                pp.tile([128, N_SEQ], BF16, tag=f"qk{m}", name=f"qk{m}")
                for m in range(3)
            ]
            qk2d = pp.tile([128, N_SEQ], BF16, tag="qk2d")
            # vt[i]: per head h: cols [128h:128h+64]=V_h, col 128h+64=ones,
            # rest zero padding (128-col stationaries engage FWL)
            vt = [
                pp.tile([128, 384], BF16, tag=f"vt{m}", name=f"vt{m}")
                for m in range(MT)
            ]
            xt = [
                pp.tile([128, N_SEQ], BF16, tag=f"xt{k}", name=f"xt{k}")
                for k in range(KT)
            ]
            wqk_t = [
                pp.tile([128, 384], BF16, tag=f"wqk{k}", name=f"wqkt{k}")
                for k in range(KT)
            ]
            wv_t = [
                pp.tile([128, 192], BF16, tag=f"wv{k}", name=f"wvt{k}")
                for k in range(KT)
            ]
            wp01_t = pp.tile([128, EMBED], BF16, tag="wp01", name="wp01t")
            wp2b_t = pp.tile([128, EMBED], BF16, tag="wp2b", name="wp2bt")
            ao01 = pp.tile([128, N_SEQ], BF16, tag="ao01", name="ao01")
            # rows 0:64 = h2 attn out; rows 64:128 = duplicate (proj pairing)
            ao2b = pp.tile([128, N_SEQ], BF16, tag="ao2b", name="ao2b")

            xT_ap = xT[:, :].rearrange("(t p) n -> t p n", p=128)
            wqk_ap = wqk[:, :].rearrange("(t p) n -> t p n", p=128)
            wv_ap = wv[:, :].rearrange("(t p) n -> t p n", p=128)
            for k in range(KT):
                nc.gpsimd.dma_start(out=wqk_t[k], in_=wqk_ap[k])
            for c in range(NDC):
                cs = slice(c * DCH, (c + 1) * DCH)
                for k in range(KT):
                    nc.gpsimd.dma_start(out=xt[k][:, cs], in_=xT_ap[k][:, cs])
            for k in range(KT):
                nc.gpsimd.dma_start(out=wv_t[k], in_=wv_ap[k])
            nc.gpsimd.dma_start(out=wp01_t, in_=wp01[:, :])
            nc.gpsimd.dma_start(out=wp2b_t, in_=wp2b[:, :])
            for m in range(MT):
                nc.gpsimd.memset(vt[m], 0.0)

            # ---- HAM warm-up: PE busy from ~1us so the clock gate is at
            # K=8/8 when the real matmuls arrive (runs during input DMA).
            wsb = pp.tile([128, QCH], BF16, tag="wsb", name="wsb")
            nc.vector.memset(wsb, 0.0)
            warm = psV.tile([128, QCH], F32, tag="pv", name="warm")
            for _ in range(30):
                nc.tensor.matmul(
                    warm, wsb[:, 0:128], wsb, start=True, stop=True,
                )

            # ---- qk generation: per (m, dchunk): one stationary per k-tile
            # serves both 512-wide halves (second LDWEIGHTS deduped).
            def qk_mtile(m, d):
                ds = slice(d * DCH, (d + 1) * DCH)
                ps = psS.tile([128, DCH], F32, tag="sS", name="psqk")
                for k in range(KT):
                    for h in range(2):
                        hs = slice(d * DCH + h * QCH, d * DCH + (h + 1) * QCH)
                        nc.tensor.matmul(
                            ps[:, h * QCH : (h + 1) * QCH],
                            wqk_t[k][:, m * 128 : (m + 1) * 128],
                            xt[k][:, hs],
                            start=(k == 0),
                            stop=(k == KT - 1),
                        )
                nc.vector.tensor_copy(out=qk[m][:, ds], in_=ps)

            for m in (1, 0, 2):
                for d in range(NDC):
                    qk_mtile(m, d)
                if m == 2:
                    # [Qh2|Kh2] -> swapped copy [Kh2|Qh2]
                    nc.gpsimd.dma_start(out=qk2d[0:64, :], in_=qk[2][64:128, :])
                    nc.gpsimd.dma_start(out=qk2d[64:128, :], in_=qk[2][0:64, :])

            # ---- exp emission: ScalarE exact or VectorE fast-exp2 ----
            def exp_emit(pt, s, dve):
                if dve:
                    nc.vector.tensor_scalar(
                        out=pt[:, :].bitcast(I16),
                        in0=s[:, :],
                        scalar1=FEXP_C1,
                        scalar2=FEXP_C2,
                        op0=MULT,
                        op1=ADD,
                    )
                else:
                    nc.scalar.activation(out=pt, in_=s, func=EXP, scale=0.125)

            # ---- scores for h0/h1: per key-tile, row-paired heads, both
            # q-halves on one stationary pair ----
            def scores01_kt(d, kt, dve1=True):
                ks = slice(kt * 128, (kt + 1) * 128)
                s0 = psS.tile([128, DCH], F32, tag="sS", name="s0")
                s1 = psS.tile([128, DCH], F32, tag="sS", name="s1")
                for h in range(2):
                    qs = slice(d * DCH + h * QCH, d * DCH + (h + 1) * QCH)
                    hs = slice(h * QCH, (h + 1) * QCH)
                    nc.tensor.matmul(
                        s0[:, hs], qk[1][0:64, ks], qk[0][0:64, qs],
                        start=True, stop=True, tile_position=(0, 0),
                    )
                    nc.tensor.matmul(
                        s1[:, hs], qk[1][64:128, ks], qk[0][64:128, qs],
                        start=True, stop=True, tile_position=(64, 0),
                    )
                pt0 = ptp.tile([128, DCH], BF16, tag="ptg", name="pt0")
                pt1 = ptp.tile([128, DCH], BF16, tag="ptg", name="pt1")
                exp_emit(pt0, s0, False)
                exp_emit(pt1, s1, dve1)
                return pt0, pt1

            # ---- scores for h2: two key-tiles row-paired via qk2d ----
            def scores2_2kt(d, g):
                i0, i1 = 2 * g, 2 * g + 1
                ksA = slice(i0 * 128, (i0 + 1) * 128)
                ksB = slice(i1 * 128, (i1 + 1) * 128)
                s2a = psS.tile([128, DCH], F32, tag="sS", name="s2a")
                s2b = psS.tile([128, DCH], F32, tag="sS", name="s2b")
                for h in range(2):
                    qs = slice(d * DCH + h * QCH, d * DCH + (h + 1) * QCH)
                    hs = slice(h * QCH, (h + 1) * QCH)
                    nc.tensor.matmul(
                        s2a[:, hs], qk2d[0:64, ksA], qk[2][0:64, qs],
                        start=True, stop=True, tile_position=(0, 0),
                    )
                    nc.tensor.matmul(
                        s2b[:, hs], qk[2][64:128, ksB], qk2d[64:128, qs],
                        start=True, stop=True, tile_position=(64, 0),
                    )
                pa = ptp.tile([128, DCH], BF16, tag="ptg", name="pt2a")
                pb = ptp.tile([128, DCH], BF16, tag="ptg", name="pt2b")
                exp_emit(pa, s2a, False)
                exp_emit(pb, s2b, True)
                return pa, pb

            # ---- attn@V for one head/key-tile: one stationary, both halves
            def attnv_kt(h, kt, pt, pvA, pvB):
                st = vt[kt][:, h * 128 : (h + 1) * 128]
                nc.tensor.matmul(
                    pvA, st, pt[:, 0:QCH],
                    start=(kt == 0), stop=(kt == MT - 1),
                )
                nc.tensor.matmul(
                    pvB, st, pt[:, QCH:],
                    start=(kt == 0), stop=(kt == MT - 1),
                )

            def softmax_divide(h, pv, qs):
                """Drain pv once (frees the PSUM bank), then normalize."""
                ov = wk.tile([65, QCH], F32, tag="ov", name="ov")
                nc.vector.tensor_copy(out=ov, in_=pv[0:65, :])
                # reciprocal with all 128 lanes: reshape [1,512] -> [128,4]
                rw = wk.tile([128, QCH // 128], F32, tag="rw", name="rw")
                nc.gpsimd.dma_start(out=rw, in_=ov[64:65, :])
                nc.vector.reciprocal(out=rw, in_=rw)
                rs0 = wk.tile([1, QCH], F32, tag="rs0", name="rs0")
                nc.gpsimd.dma_start(out=rs0, in_=rw)
                bc = wk.tile([64, QCH], F32, tag="bc", name="bc")
                nc.gpsimd.partition_broadcast(bc, rs0)
                if h == 0:
                    dst = ao01[0:64, qs]
                elif h == 1:
                    dst = ao01[64:128, qs]
                else:
                    dst = ao2b[0:64, qs]
                nc.vector.tensor_mul(out=dst, in0=ov[0:64, :], in1=bc)
                if h == 2:
                    # duplicate h2 rows onto partitions 64-127 for the
                    # row-paired proj matmuls (cross-partition -> DMA)
                    nc.gpsimd.dma_start(out=ao2b[64:128, qs], in_=ao2b[0:64, qs])

            # ---- V in natural layout [seq, ch] ----
            def vnat_mtile(m):
                ps = psV.tile([128, 192], F32, tag="pv", name="psv")
                for k in range(KT):
                    nc.tensor.matmul(
                        ps,
                        xt[k][:, m * 128 : (m + 1) * 128],
                        wv_t[k],
                        start=(k == 0),
                        stop=(k == KT - 1),
                    )
                for h in range(N_HEADS_CORE):
                    nc.vector.tensor_copy(
                        out=vt[m][:, 128 * h : 128 * h + 64],
                        in_=ps[:, 64 * h : 64 * h + 64],
                    )
                nc.gpsimd.dma_start(
                    out=vt[m].rearrange("p (h c) -> p h c", c=128)[:, :, 64],
                    in_=ones[:, :],
                )

            # ---- projection: per (m-pair, q-half): wp01 K=128 matmuls plus
            # row-paired wp2 K=64 matmuls for m-even/m-odd ----
            def proj_mpair(mp, d):
                m0, m1 = 2 * mp, 2 * mp + 1
                ds = slice(d * DCH, (d + 1) * DCH)
                qsA = slice(d * DCH, d * DCH + QCH)
                qsB = slice(d * DCH + QCH, (d + 1) * DCH)
                out_ap = outT[:, :].rearrange("(t p) n -> t p n", p=128)
                po0 = psS.tile([128, DCH], F32, tag="sS", name="po0")
                po1A = psV.tile([128, QCH], F32, tag="pv", name="po1A")
                po1B = psV.tile([128, QCH], F32, tag="pv", name="po1B")
                ms0 = slice(m0 * 128, (m0 + 1) * 128)
                ms1 = slice(m1 * 128, (m1 + 1) * 128)
                nc.tensor.matmul(
                    po0[:, 0:QCH], wp01_t[:, ms0], ao01[:, qsA],
                    start=True, stop=False,
                )
                nc.tensor.matmul(
                    po0[:, QCH:], wp01_t[:, ms0], ao01[:, qsB],
                    start=True, stop=False,
                )
                nc.tensor.matmul(
                    po1A, wp01_t[:, ms1], ao01[:, qsA], start=True, stop=False,
                )
                nc.tensor.matmul(
                    po1B, wp01_t[:, ms1], ao01[:, qsB], start=True, stop=False,
                )
                nc.tensor.matmul(
                    po0[:, 0:QCH], wp2b_t[0:64, ms0], ao2b[0:64, qsA],
                    start=False, stop=True, tile_position=(0, 0),
                )
                nc.tensor.matmul(
                    po1A, wp2b_t[64:128, ms1], ao2b[64:128, qsA],
                    start=False, stop=True, tile_position=(64, 0),
                )
                nc.tensor.matmul(
                    po0[:, QCH:], wp2b_t[0:64, ms0], ao2b[0:64, qsB],
                    start=False, stop=True, tile_position=(0, 0),
                )
                nc.tensor.matmul(
                    po1B, wp2b_t[64:128, ms1], ao2b[64:128, qsB],
                    start=False, stop=True, tile_position=(64, 0),
                )
                ot0 = wk.tile([128, DCH], BF16, tag="ot", name="ot0")
                nc.scalar.copy(out=ot0, in_=po0)
                nc.sync.dma_start(out=out_ap[m0][:, ds], in_=ot0)
                ot1 = wk.tile([128, DCH], BF16, tag="ot", name="ot1")
                nc.vector.tensor_copy(out=ot1[:, 0:QCH], in_=po1A)
                nc.vector.tensor_copy(out=ot1[:, QCH:], in_=po1B)
                nc.sync.dma_start(out=out_ap[m1][:, ds], in_=ot1)

            # ---- initial phase: scores01(d0) interleaved with vnat ----
            pt01 = {}
            pt2 = {}
            for kt in range(MT):
                pt01[(0, kt)] = scores01_kt(0, kt, dve1=(kt % 2 == 0))
                vnat_mtile(kt)

            # ---- main dchunk loop ----
            for d in range(NDC):
                qsA = slice(d * DCH, d * DCH + QCH)
                qsB = slice(d * DCH + QCH, (d + 1) * DCH)
                # loop 1: attnv for h0/h1 + scores2(d)
                pv0A = psV.tile([128, QCH], F32, tag="pv", name="pv0A")
                pv0B = psV.tile([128, QCH], F32, tag="pv", name="pv0B")
                pv1A = psV.tile([128, QCH], F32, tag="pv", name="pv1A")
                pv1B = psV.tile([128, QCH], F32, tag="pv", name="pv1B")
                l1_s01 = [0, 3, 6, 9, 12]
                for kt in range(MT):
                    p0, p1 = pt01.pop((d, kt))
                    attnv_kt(0, kt, p0, pv0A, pv0B)
                    attnv_kt(1, kt, p1, pv1A, pv1B)
                    if kt % 2 == 1:
                        g = kt // 2
                        pt2[(d, 2 * g)], pt2[(d, 2 * g + 1)] = scores2_2kt(d, g)
                    if d + 1 < NDC and kt in (2, 4, 7, 10, 13):
                        skt = l1_s01.pop(0)
                        pt01[(d + 1, skt)] = scores01_kt(d + 1, skt)
                softmax_divide(0, pv0A, qsA)
                softmax_divide(0, pv0B, qsB)
                softmax_divide(1, pv1A, qsA)
                softmax_divide(1, pv1B, qsB)

                # loop 2: attnv h2 + scores01(d+1) + proj(d-1)
                pv2A = psV.tile([128, QCH], F32, tag="pv", name="pv2A")
                pv2B = psV.tile([128, QCH], F32, tag="pv", name="pv2B")
                for kt in range(MT):
                    attnv_kt(2, kt, pt2.pop((d, kt)), pv2A, pv2B)
                    if d + 1 < NDC and kt not in (0, 3, 6, 9, 12):
                        pt01[(d + 1, kt)] = scores01_kt(d + 1, kt)
                    if d == 1 and kt in (4, 9, 14):
                        proj_mpair({4: 0, 9: 1, 14: 2}[kt], 0)
                softmax_divide(2, pv2A, qsA)
                softmax_divide(2, pv2B, qsB)

            # keep the PE clock gate open across the divide chain that
            # precedes the epilogue projection
            tailw = psS.tile([128, QCH], F32, tag="sS", name="tailw")
            for _ in range(24):
                nc.tensor.matmul(
                    tailw, wsb[:, 0:128], wsb, start=True, stop=True,
                )

            # epilogue: last dchunk's projection
            for mp in range(3):
                proj_mpair(mp, 1)

    ndel = _ldw_dedup(nc)
    nc.compile()
    nc._ldw_deduped = ndel
    return nc


def _get_nc():
    if "nc" not in _CACHED:
        _CACHED["nc"] = _build()
    return _CACHED["nc"]


def _shard_inputs(x, w_qkv, w_proj):
    """Build the 8 per-core input maps (bf16 operands)."""
    import ml_dtypes

    bf = ml_dtypes.bfloat16
    in_maps = []
    for core in range(N_CORES):
        b = core // 4
        h0 = 3 * (core % 4)
        heads = [h0, h0 + 1, h0 + 2]
        xTc = np.ascontiguousarray(x[b].T).astype(bf)
        wq = [w_qkv[:, h * HD : (h + 1) * HD] for h in heads]
        wk_ = [w_qkv[:, EMBED + h * HD : EMBED + (h + 1) * HD] for h in heads]
        wv_ = [
            w_qkv[:, 2 * EMBED + h * HD : 2 * EMBED + (h + 1) * HD] for h in heads
        ]
        wqk = np.concatenate(
            [wq[0], wq[1], wk_[0], wk_[1], wq[2], wk_[2]], axis=1
        ).astype(bf)
        wvp = np.concatenate([wv_[0], wv_[1], wv_[2]], axis=1).astype(bf)
        wps = [
            np.ascontiguousarray(w_proj[h * HD : (h + 1) * HD, :]).astype(bf)
            for h in heads
        ]
        wp2b = np.concatenate([wps[2], wps[2]], axis=0)
        in_maps.append(
            {
                "ones": np.ones((128, 3), bf),
                "xT": xTc,
                "wqk": np.ascontiguousarray(wqk),
                "wv": np.ascontiguousarray(wvp),
                "wp01": np.ascontiguousarray(np.concatenate([wps[0], wps[1]], axis=0)),
                "wp2b": np.ascontiguousarray(wp2b),
            }
        )
    return in_maps


def kernel(x, w_qkv, w_proj, _trace=False):
    from concourse.bass_utils import run_bass_kernel_spmd

    x = np.asarray(x, dtype=np.float32)
    w_qkv = np.asarray(w_qkv, dtype=np.float32)
    w_proj = np.asarray(w_proj, dtype=np.float32)

    nc = _get_nc()
    in_maps = _shard_inputs(x, w_qkv, w_proj)
    res = run_bass_kernel_spmd(
        nc, in_maps, core_ids=list(range(N_CORES)), trace=_trace
    )
    _CACHED["last_results"] = res

    out = np.empty((2, N_SEQ, EMBED), dtype=np.float32)
    for b in range(2):
        acc = res.results[4 * b]["outT"].astype(np.float32).copy()
        for g in range(1, 4):
            acc += res.results[4 * b + g]["outT"].astype(np.float32)
        out[b] = acc.T
    return out


# revision 10
# speedup vs baseline: 1.0140x; 1.0068x over previous
"""Multi-head self-attention Trainium2 kernel (v2).

Problem: x[2, 2048, 768] -> MHSA (12 heads, head_dim 64) -> out[2, 2048, 768].

Sharding over 8 NeuronCores: core c handles batch c//4 and heads
[3*(c%4), 3*(c%4)+3). Each core computes its 3 heads' attention and a
row-split partial of the output projection over its 192 channels; the host
sums the 4 partials per batch and transposes.

v2 structure: queries are processed in two 1024-wide double-chunks so every
matmul stationary serves two 512-column moving passes; a pre-compile IR pass
then deletes the redundant second LDWEIGHTS of each pair (the Tile framework
emits one per matmul), removing ~100ns of serialized weight-load per matmul.
V stationaries are padded to 128 columns ([V|ones|0...]) to engage FWL.
The softmax exp is split between ScalarE (exact LUT exp) and VectorE
(Schraudolph fast-exp2: bits = round(s*C1 + C2) written as int16, bitcast to
bf16); the ~2% relative error on the DVE share is attenuated ~sqrt(neff)~27x
by softmax averaging. Row-sums ride the attn@V matmul as a ones column.
The output projection contracts h0/h1 as K=128 matmuls plus row-paired K=64
h2 matmuls (wp2 duplicated on partitions 64-127, m-even/odd pairs run
concurrently); it runs inside the second group loop of the last dchunk and
the epilogue. Matmul operands are bf16 (fp32 PSUM); normalization is fp32.
"""

import sys

sys.path.insert(0, "/opt/trn_rl_repo")

import numpy as np

EMBED = 768
N_SEQ = 2048
N_HEADS_CORE = 3
HD = 64
N_CORES = 8
KT = EMBED // 128  # 6 contraction tiles for the projections
MT = N_SEQ // 128  # 16 key-row tiles
QCH = 512  # PSUM-bank query chunk
DCH = 1024  # stationary-reuse double chunk
NDC = N_SEQ // DCH  # 2

# fast-exp2 constants: bf16 bits = trunc(s*0.125*log2e*128 + C2)
FEXP_C1 = 0.125 * 1.4426950408889634 * 128.0
FEXP_C2 = 127.0 * 128.0 + 0.5 - 5.5

_CACHED = {}


def _ldw_dedup(nc):
    """Delete InstLdweights whose weights are already resident.

    Tracks the last LDWEIGHTS per PE-array region (tile_position + extent);
    an LDWEIGHTS identical to the live one for a non-overwritten region is
    redundant. Dependency references to deleted instructions are remapped to
    the surviving load.
    """

    def ap_key(ins):
        ap = ins.ins[0]
        return (
            str(ap.memref),
            int(ap.offset),
            str(ap.ap),
            str(ap.dtype),
            str(ins.perf_mode),
            str(ins.is_transpose),
        )

    def extent(ins):
        pos = ins.tile_position
        if pos is None:
            return (0, 128, 0, 128)
        ap = ins.ins[0]
        try:
            pairs = list(ap.ap)
            rows = int(pairs[0][1])
            cols = int(pairs[-1][1]) if len(pairs) > 1 else 128
        except Exception:
            rows, cols = 128, 128
        r0, c0 = int(pos[0]), int(pos[1])
        return (r0, min(128, r0 + rows), c0, min(128, c0 + cols))

    removed = 0
    for f in nc.m.functions:
        for b in f.blocks:
            live = {}  # (r0,r1,c0,c1) -> (key, name)
            keep = []
            remap = {}
            for ins in b.instructions:
                if type(ins).__name__ == "InstLdweights":
                    k = ap_key(ins)
                    ext = extent(ins)
                    prev = live.get(ext)
                    if prev is not None and prev[0] == k:
                        remap[ins.name] = prev[1]
                        removed += 1
                        continue
                    # evict overlapping regions
                    r0, r1, c0, c1 = ext
                    for e in list(live):
                        if e != ext and not (
                            e[1] <= r0 or e[0] >= r1 or e[3] <= c0 or e[2] >= c1
                        ):
                            del live[e]
                    live[ext] = (k, ins.name)
                keep.append(ins)
            if remap:
                for ins in keep:
                    ins.remap_dependency_names(remap)
                b.instructions[:] = keep
    return removed


def _build():
    from concourse import bacc
    import concourse.tile as tile
    import concourse.mybir as mybir

    F32 = mybir.dt.float32
    BF16 = mybir.dt.bfloat16
    I16 = mybir.dt.int16
    EXP = mybir.ActivationFunctionType.Exp
    MULT = mybir.AluOpType.mult
    ADD = mybir.AluOpType.add

    nc = bacc.Bacc()
    xT = nc.declare_dram_parameter("xT", [EMBED, N_SEQ], BF16, isOutput=False)
    # Q/K weights in 3 m-tiles of 128 cols: [Qh0|Qh1], [Kh0|Kh1], [Qh2|Kh2]
    wqk = nc.declare_dram_parameter("wqk", [EMBED, 384], BF16, isOutput=False)
    wv = nc.declare_dram_parameter("wv", [EMBED, 192], BF16, isOutput=False)
    wp01 = nc.declare_dram_parameter("wp01", [128, EMBED], BF16, isOutput=False)
    # wp2 duplicated on both partition halves for row-paired h2 proj matmuls
    wp2b = nc.declare_dram_parameter("wp2b", [128, EMBED], BF16, isOutput=False)
    ones = nc.declare_dram_parameter("ones", [128, 3], BF16, isOutput=False)
    outT = nc.declare_dram_parameter("outT", [EMBED, N_SEQ], BF16, isOutput=True)

    with tile.TileContext(nc) as tc:
        with (
            tc.tile_pool(name="persist", bufs=1) as pp,
            tc.tile_pool(name="pt", bufs=36) as ptp,
            tc.tile_pool(name="work", bufs=5) as wk,
            tc.tile_pool(name="psS", bufs=2, space="PSUM") as psS,
            tc.tile_pool(name="psV", bufs=4, space="PSUM") as psV,
        ):
            qk = [
                pp.tile([128, N_SEQ], BF16, tag=f"qk{m}", name=f"qk{m}")
                for m in range(3)
            ]
            qk2d = pp.tile([128, N_SEQ], BF16, tag="qk2d")
            # vt[i]: per head h: cols [128h:128h+64]=V_h, col 128h+64=ones,
            # rest zero padding (128-col stationaries engage FWL)
            vt = [
                pp.tile([128, 384], BF16, tag=f"vt{m}", name=f"vt{m}")
                for m in range(MT)
            ]
            xt = [
                pp.tile([128, N_SEQ], BF16, tag=f"xt{k}", name=f"xt{k}")
                for k in range(KT)
            ]
            wqk_t = [
                pp.tile([128, 384], BF16, tag=f"wqk{k}", name=f"wqkt{k}")
                for k in range(KT)
            ]
            wv_t = [
                pp.tile([128, 192], BF16, tag=f"wv{k}", name=f"wvt{k}")
                for k in range(KT)
            ]
            wp01_t = pp.tile([128, EMBED], BF16, tag="wp01", name="wp01t")
            wp2b_t = pp.tile([128, EMBED], BF16, tag="wp2b", name="wp2bt")
            ao01 = pp.tile([128, N_SEQ], BF16, tag="ao01", name="ao01")
            # rows 0:64 = h2 attn out; rows 64:128 = duplicate (proj pairing)
            ao2b = pp.tile([128, N_SEQ], BF16, tag="ao2b", name="ao2b")

            xT_ap = xT[:, :].rearrange("(t p) n -> t p n", p=128)
            wqk_ap = wqk[:, :].rearrange("(t p) n -> t p n", p=128)
            wv_ap = wv[:, :].rearrange("(t p) n -> t p n", p=128)
            for k in range(KT):
                nc.gpsimd.dma_start(out=wqk_t[k], in_=wqk_ap[k])
            for c in range(NDC):
                cs = slice(c * DCH, (c + 1) * DCH)
                for k in range(KT):
                    nc.gpsimd.dma_start(out=xt[k][:, cs], in_=xT_ap[k][:, cs])
            for k in range(KT):
                nc.gpsimd.dma_start(out=wv_t[k], in_=wv_ap[k])
            nc.gpsimd.dma_start(out=wp01_t, in_=wp01[:, :])
            nc.gpsimd.dma_start(out=wp2b_t, in_=wp2b[:, :])
            for m in range(MT):
                nc.gpsimd.memset(vt[m], 0.0)

            # ---- HAM warm-up: PE busy from ~1us so the clock gate is at
            # K=8/8 when the real matmuls arrive (runs during input DMA).
            wsb = pp.tile([128, QCH], BF16, tag="wsb", name="wsb")
            nc.vector.memset(wsb, 0.0)
            warm = psV.tile([128, QCH], F32, tag="pv", name="warm")
            for _ in range(30):
                nc.tensor.matmul(
                    warm, wsb[:, 0:128], wsb, start=True, stop=True,
                )

            # ---- qk generation: per (m, dchunk): one stationary per k-tile
            # serves both 512-wide halves (second LDWEIGHTS deduped).
            def qk_mtile(m, d):
                ds = slice(d * DCH, (d + 1) * DCH)
                ps = psS.tile([128, DCH], F32, tag="sS", name="psqk")
                for k in range(KT):
                    for h in range(2):
                        hs = slice(d * DCH + h * QCH, d * DCH + (h + 1) * QCH)
                        nc.tensor.matmul(
                            ps[:, h * QCH : (h + 1) * QCH],
                            wqk_t[k][:, m * 128 : (m + 1) * 128],
                            xt[k][:, hs],
                            start=(k == 0),
                            stop=(k == KT - 1),
                        )
                nc.vector.tensor_copy(out=qk[m][:, ds], in_=ps)

            for m in (1, 0, 2):
                for d in range(NDC):
                    qk_mtile(m, d)
                if m == 2:
                    # [Qh2|Kh2] -> swapped copy [Kh2|Qh2]
                    nc.gpsimd.dma_start(out=qk2d[0:64, :], in_=qk[2][64:128, :])
                    nc.gpsimd.dma_start(out=qk2d[64:128, :], in_=qk[2][0:64, :])

            # ---- exp emission: ScalarE exact or VectorE fast-exp2 ----
            def exp_emit(pt, s, dve):
                if dve == "pool":
                    nc.gpsimd.tensor_scalar(
                        out=pt[:, :].bitcast(I16),
                        in0=s[:, :],
                        scalar1=FEXP_C1,
                        scalar2=FEXP_C2,
                        op0=MULT,
                        op1=ADD,
                    )
                    return
                if dve:
                    nc.vector.tensor_scalar(
                        out=pt[:, :].bitcast(I16),
                        in0=s[:, :],
                        scalar1=FEXP_C1,
                        scalar2=FEXP_C2,
                        op0=MULT,
                        op1=ADD,
                    )
                else:
                    nc.scalar.activation(out=pt, in_=s, func=EXP, scale=0.125)

            # ---- scores for h0/h1: per key-tile, row-paired heads, both
            # q-halves on one stationary pair ----
            def scores01_kt(d, kt, dve1=True):
                ks = slice(kt * 128, (kt + 1) * 128)
                s0 = psS.tile([128, DCH], F32, tag="sS", name="s0")
                s1 = psS.tile([128, DCH], F32, tag="sS", name="s1")
                for h in range(2):
                    qs = slice(d * DCH + h * QCH, d * DCH + (h + 1) * QCH)
                    hs = slice(h * QCH, (h + 1) * QCH)
                    nc.tensor.matmul(
                        s0[:, hs], qk[1][0:64, ks], qk[0][0:64, qs],
                        start=True, stop=True, tile_position=(0, 0),
                    )
                    nc.tensor.matmul(
                        s1[:, hs], qk[1][64:128, ks], qk[0][64:128, qs],
                        start=True, stop=True, tile_position=(64, 0),
                    )
                pt0 = ptp.tile([128, DCH], BF16, tag="ptg", name="pt0")
                pt1 = ptp.tile([128, DCH], BF16, tag="ptg", name="pt1")
                exp_emit(pt0, s0, False)
                exp_emit(pt1, s1, dve1)
                return pt0, pt1

            # ---- scores for h2: two key-tiles row-paired via qk2d ----
            def scores2_2kt(d, g):
                i0, i1 = 2 * g, 2 * g + 1
                ksA = slice(i0 * 128, (i0 + 1) * 128)
                ksB = slice(i1 * 128, (i1 + 1) * 128)
                s2a = psS.tile([128, DCH], F32, tag="sS", name="s2a")
                s2b = psS.tile([128, DCH], F32, tag="sS", name="s2b")
                for h in range(2):
                    qs = slice(d * DCH + h * QCH, d * DCH + (h + 1) * QCH)
                    hs = slice(h * QCH, (h + 1) * QCH)
                    nc.tensor.matmul(
                        s2a[:, hs], qk2d[0:64, ksA], qk[2][0:64, qs],
                        start=True, stop=True, tile_position=(0, 0),
                    )
                    nc.tensor.matmul(
                        s2b[:, hs], qk[2][64:128, ksB], qk2d[64:128, qs],
                        start=True, stop=True, tile_position=(64, 0),
                    )
                pa = ptp.tile([128, DCH], BF16, tag="ptg", name="pt2a")
                pb = ptp.tile([128, DCH], BF16, tag="ptg", name="pt2b")
                exp_emit(pa, s2a, False)
                exp_emit(pb, s2b, True)
                return pa, pb

            # ---- attn@V for one head/key-tile: one stationary, both halves
            def attnv_kt(h, kt, pt, pvA, pvB):
                st = vt[kt][:, h * 128 : (h + 1) * 128]
                nc.tensor.matmul(
                    pvA, st, pt[:, 0:QCH],
                    start=(kt == 0), stop=(kt == MT - 1),
                )
                nc.tensor.matmul(
                    pvB, st, pt[:, QCH:],
                    start=(kt == 0), stop=(kt == MT - 1),
                )

            def softmax_divide(h, pv, qs):
                """Drain pv once (frees the PSUM bank), then normalize."""
                ov = wk.tile([65, QCH], F32, tag="ov", name="ov")
                nc.scalar.copy(out=ov, in_=pv[0:65, :])
                # reciprocal with all 128 lanes: reshape [1,512] -> [128,4]
                rw = wk.tile([128, QCH // 128], F32, tag="rw", name="rw")
                nc.gpsimd.dma_start(out=rw, in_=ov[64:65, :])
                nc.vector.reciprocal(out=rw, in_=rw)
                rs0 = wk.tile([1, QCH], F32, tag="rs0", name="rs0")
                nc.gpsimd.dma_start(out=rs0, in_=rw)
                bc = wk.tile([64, QCH], F32, tag="bc", name="bc")
                nc.gpsimd.partition_broadcast(bc, rs0)
                if h == 0:
                    dst = ao01[0:64, qs]
                elif h == 1:
                    dst = ao01[64:128, qs]
                else:
                    dst = ao2b[0:64, qs]
                nc.vector.tensor_mul(out=dst, in0=ov[0:64, :], in1=bc)
                if h == 2:
                    # duplicate h2 rows onto partitions 64-127 for the
                    # row-paired proj matmuls (cross-partition -> DMA)
                    nc.gpsimd.dma_start(out=ao2b[64:128, qs], in_=ao2b[0:64, qs])

            # ---- V in natural layout [seq, ch] ----
            def vnat_mtile(m):
                ps = psV.tile([128, 192], F32, tag="pv", name="psv")
                for k in range(KT):
                    nc.tensor.matmul(
                        ps,
                        xt[k][:, m * 128 : (m + 1) * 128],
                        wv_t[k],
                        start=(k == 0),
                        stop=(k == KT - 1),
                    )
                for h in range(N_HEADS_CORE):
                    nc.vector.tensor_copy(
                        out=vt[m][:, 128 * h : 128 * h + 64],
                        in_=ps[:, 64 * h : 64 * h + 64],
                    )
                nc.gpsimd.dma_start(
                    out=vt[m].rearrange("p (h c) -> p h c", c=128)[:, :, 64],
                    in_=ones[:, :],
                )

            # ---- projection: per (m-pair, q-half): wp01 K=128 matmuls plus
            # row-paired wp2 K=64 matmuls for m-even/m-odd ----
            def proj_mpair(mp, d):
                m0, m1 = 2 * mp, 2 * mp + 1
                ds = slice(d * DCH, (d + 1) * DCH)
                qsA = slice(d * DCH, d * DCH + QCH)
                qsB = slice(d * DCH + QCH, (d + 1) * DCH)
                out_ap = outT[:, :].rearrange("(t p) n -> t p n", p=128)
                po0 = psS.tile([128, DCH], F32, tag="sS", name="po0")
                po1A = psV.tile([128, QCH], F32, tag="pv", name="po1A")
                po1B = psV.tile([128, QCH], F32, tag="pv", name="po1B")
                ms0 = slice(m0 * 128, (m0 + 1) * 128)
                ms1 = slice(m1 * 128, (m1 + 1) * 128)
                nc.tensor.matmul(
                    po0[:, 0:QCH], wp01_t[:, ms0], ao01[:, qsA],
                    start=True, stop=False,
                )
                nc.tensor.matmul(
                    po0[:, QCH:], wp01_t[:, ms0], ao01[:, qsB],
                    start=True, stop=False,
                )
                nc.tensor.matmul(
                    po1A, wp01_t[:, ms1], ao01[:, qsA], start=True, stop=False,
                )
                nc.tensor.matmul(
                    po1B, wp01_t[:, ms1], ao01[:, qsB], start=True, stop=False,
                )
                nc.tensor.matmul(
                    po0[:, 0:QCH], wp2b_t[0:64, ms0], ao2b[0:64, qsA],
                    start=False, stop=True, tile_position=(0, 0),
                )
                nc.tensor.matmul(
                    po1A, wp2b_t[64:128, ms1], ao2b[64:128, qsA],
                    start=False, stop=True, tile_position=(64, 0),
                )
                nc.tensor.matmul(
                    po0[:, QCH:], wp2b_t[0:64, ms0], ao2b[0:64, qsB],
                    start=False, stop=True, tile_position=(0, 0),
                )
                nc.tensor.matmul(
                    po1B, wp2b_t[64:128, ms1], ao2b[64:128, qsB],
                    start=False, stop=True, tile_position=(64, 0),
                )
                ot0 = wk.tile([128, DCH], BF16, tag="ot", name="ot0")
                nc.scalar.copy(out=ot0, in_=po0)
                nc.sync.dma_start(out=out_ap[m0][:, ds], in_=ot0)
                ot1 = wk.tile([128, DCH], BF16, tag="ot", name="ot1")
                nc.vector.tensor_copy(out=ot1[:, 0:QCH], in_=po1A)
                nc.vector.tensor_copy(out=ot1[:, QCH:], in_=po1B)
                nc.sync.dma_start(out=out_ap[m1][:, ds], in_=ot1)

            # ---- initial phase: scores01(d0) interleaved with vnat ----
            pt01 = {}
            pt2 = {}
            for kt in range(MT):
                pt01[(0, kt)] = scores01_kt(0, kt, dve1=(kt % 2 == 0))
                vnat_mtile(kt)

            # ---- main dchunk loop ----
            for d in range(NDC):
                qsA = slice(d * DCH, d * DCH + QCH)
                qsB = slice(d * DCH + QCH, (d + 1) * DCH)
                # loop 1: attnv for h0/h1 + scores2(d)
                pv1A = psV.tile([128, QCH], F32, tag="pv", name="pv1A")
                pv1B = psV.tile([128, QCH], F32, tag="pv", name="pv1B")
                pv0A = psV.tile([128, QCH], F32, tag="pv", name="pv0A")
                pv0B = psV.tile([128, QCH], F32, tag="pv", name="pv0B")
                l1_s01 = [0, 3, 6, 9, 12]
                for kt in range(MT):
                    p0, p1 = pt01.pop((d, kt))
                    attnv_kt(1, kt, p1, pv1A, pv1B)
                    attnv_kt(0, kt, p0, pv0A, pv0B)
                    if kt % 2 == 1:
                        g = kt // 2
                        pt2[(d, 2 * g)], pt2[(d, 2 * g + 1)] = scores2_2kt(d, g)
                    if d + 1 < NDC and kt in (2, 4, 7, 10, 13):
                        skt = l1_s01.pop(0)
                        pt01[(d + 1, skt)] = scores01_kt(d + 1, skt)
                softmax_divide(0, pv0A, qsA)
                softmax_divide(0, pv0B, qsB)
                softmax_divide(1, pv1A, qsA)
                softmax_divide(1, pv1B, qsB)

                # loop 2: attnv h2 + scores01(d+1) + proj(d-1)
                pv2A = psV.tile([128, QCH], F32, tag="pv", name="pv2A")
                pv2B = psV.tile([128, QCH], F32, tag="pv", name="pv2B")
                for kt in range(MT):
                    attnv_kt(2, kt, pt2.pop((d, kt)), pv2A, pv2B)
                    if d + 1 < NDC and kt not in (0, 3, 6, 9, 12):
                        pt01[(d + 1, kt)] = scores01_kt(d + 1, kt)
                    if d == 1 and kt in (4, 9, 14):
                        proj_mpair({4: 0, 9: 1, 14: 2}[kt], 0)
                softmax_divide(2, pv2A, qsA)
                softmax_divide(2, pv2B, qsB)

            # keep the PE clock gate open across the divide chain that
            # precedes the epilogue projection
            tailw = psS.tile([128, QCH], F32, tag="sS", name="tailw")
            for _ in range(24):
                nc.tensor.matmul(
                    tailw, wsb[:, 0:128], wsb, start=True, stop=True,
                )

            # epilogue: last dchunk's projection
            for mp in range(3):
                proj_mpair(mp, 1)

    ndel = _ldw_dedup(nc)
    nc.compile()
    nc._ldw_deduped = ndel
    return nc


def _get_nc():
    if "nc" not in _CACHED:
        _CACHED["nc"] = _build()
    return _CACHED["nc"]


def _shard_inputs(x, w_qkv, w_proj):
    """Build the 8 per-core input maps (bf16 operands)."""
    import ml_dtypes

    bf = ml_dtypes.bfloat16
    in_maps = []
    for core in range(N_CORES):
        b = core // 4
        h0 = 3 * (core % 4)
        heads = [h0, h0 + 1, h0 + 2]
        xTc = np.ascontiguousarray(x[b].T).astype(bf)
        wq = [w_qkv[:, h * HD : (h + 1) * HD] for h in heads]
        wk_ = [w_qkv[:, EMBED + h * HD : EMBED + (h + 1) * HD] for h in heads]
        wv_ = [
            w_qkv[:, 2 * EMBED + h * HD : 2 * EMBED + (h + 1) * HD] for h in heads
        ]
        wqk = np.concatenate(
            [wq[0], wq[1], wk_[0], wk_[1], wq[2], wk_[2]], axis=1
        ).astype(bf)
        wvp = np.concatenate([wv_[0], wv_[1], wv_[2]], axis=1).astype(bf)
        wps = [
            np.ascontiguousarray(w_proj[h * HD : (h + 1) * HD, :]).astype(bf)
            for h in heads
        ]
        wp2b = np.concatenate([wps[2], wps[2]], axis=0)
        in_maps.append(
            {
                "ones": np.ones((128, 3), bf),
                "xT": xTc,
                "wqk": np.ascontiguousarray(wqk),
                "wv": np.ascontiguousarray(wvp),
                "wp01": np.ascontiguousarray(np.concatenate([wps[0], wps[1]], axis=0)),
                "wp2b": np.ascontiguousarray(wp2b),
            }
        )
    return in_maps


def kernel(x, w_qkv, w_proj, _trace=False):
    from concourse.bass_utils import run_bass_kernel_spmd

    x = np.asarray(x, dtype=np.float32)
    w_qkv = np.asarray(w_qkv, dtype=np.float32)
    w_proj = np.asarray(w_proj, dtype=np.float32)

    nc = _get_nc()
    in_maps = _shard_inputs(x, w_qkv, w_proj)
    res = run_bass_kernel_spmd(
        nc, in_maps, core_ids=list(range(N_CORES)), trace=_trace
    )
    _CACHED["last_results"] = res

    out = np.empty((2, N_SEQ, EMBED), dtype=np.float32)
    for b in range(2):
        acc = res.results[4 * b]["outT"].astype(np.float32).copy()
        for g in range(1, 4):
            acc += res.results[4 * b + g]["outT"].astype(np.float32)
        out[b] = acc.T
    return out
